# revision 12
# baseline (speedup 1.0000x reference)
"""BinaryOneToManyMatcher (nms_detection) Trainium2 Bass kernel.

Computes, for B=128 images with Q=1000 predicted boxes and G=300 GT boxes:
  score = sigmoid(pred_logits)            [B,Q]
  iou   = pairwise IoU(pred, tgt)         [B,Q,G]
  gt    = score * iou * (iou > 0.4)       [B,Q,G]
  vals, idxs = top_k(gt over Q, k=4); mask = vals > 0

Sharding: pure data parallel, 16 images per NeuronCore across 8 cores.

Per-core layout: for each image, G on partitions (chunks of <=128) and Q on
the free dim (1000 wide).  Per-query rows (x1,y1,x2,y2,area,score) are
broadcast across partitions with stride-0 DMA; per-target values are [P,1]
per-partition scalars, so the whole IoU chain runs as fused
tensor_scalar / scalar_tensor_tensor ops on the Vector engine.

Top-4 uses the DVE Max8 instruction (top-8 per partition, descending) +
MaxIndex.  A strictly-decreasing per-q bias of scale 2^-40 is added to the
masked scores so every value in a row is distinct; this makes tie handling
exact: zero entries (invalid pairs) sort by ascending q, matching
jax.lax.top_k's lowest-index-first tie rule, and the bias is far below the
minimum positive score gap so positive ordering is unchanged.
"""

from contextlib import ExitStack

import numpy as np

import concourse.bass as bass
import concourse.tile as tile
from concourse import bacc, mybir
from concourse.bass_utils import run_bass_kernel_spmd

B, Q, G, K = 128, 1000, 300, 4
NCORES = 8
BPC = B // NCORES  # images per core

F32 = mybir.dt.float32
I32 = mybir.dt.int32
U32 = mybir.dt.uint32
U8 = mybir.dt.uint8
Op = mybir.AluOpType

BIAS_SCALE = float(2.0**-40)  # per-q tie-break bias scale
POS_THRESH = 1e-6  # separates real positives (>=3e-3) from bias values (<1e-9)


def _register_wsub():
    """Custom DVE op: out = min(in0, s0) - max(in1, s1) in one pass.

    Computes the overlap width rb-lt of the IoU kernel (normally a
    tensor_scalar max + a fused min/subtract = 2 DVE passes) in a single
    full-rate instruction.  Rounding matches the reference exactly: min/max
    are exact, one rounded subtract.
    """
    from concourse import dve_ops
    from concourse.dve_spec import Spec, Src0, Src1, C0, C1, minn, maxx, lower
    from concourse.dve_uop import DveOpSpec

    for op in dve_ops.OPS:
        if op.name == "WSUB_ANT":
            return op

    spec = Spec(
        body=minn(Src0, C0) - maxx(Src1, C1),
        reference=lambda in0, in1, s0, s1, imm2: (
            np.minimum(in0.astype(np.float32), s0) - np.maximum(in1, s1)
        ).astype(np.float32),
    )
    shas = {}
    for ver in ("v3", "v4"):
        try:
            uops = lower(spec, ver=ver)
            shas[ver] = DveOpSpec(
                name="WSUB_ANT", opcode=0, uops=uops, rd1_en=True
            ).sha(ver)
        except Exception:
            pass
    op = dve_ops.DveOp("WSUB_ANT", spec, subdim=False, uops_sha=shas)
    dve_ops.OPS.append(op)
    dve_ops.CUSTOM_DVE_SPECS[op.name] = spec
    dve_ops._SUB_OPCODE_FOR_NAME[op.name] = (
        max(dve_ops._SUB_OPCODE_FOR_NAME.values()) + 1
    )
    assert dve_ops._SUB_OPCODE_FOR_NAME[op.name] < 0x20
    return op


def _build_kernel(reps=1):
    wsub = _register_wsub()
    nc = bacc.Bacc("TRN2", target_bir_lowering=False, debug=False,
                   num_devices=NCORES)

    pl = nc.dram_tensor("pred_logits", [BPC, Q, 1], F32, kind="ExternalInput").ap()
    pb = nc.dram_tensor("pred_boxes", [BPC, Q, 4], F32, kind="ExternalInput").ap()
    tb = nc.dram_tensor("tgt_boxes", [BPC, G, 4], F32, kind="ExternalInput").ap()

    vals_o = nc.dram_tensor("vals", [BPC, G, K], F32, kind="ExternalOutput").ap()
    idxs_o = nc.dram_tensor("idxs", [BPC, G, K], I32, kind="ExternalOutput").ap()
    mask_o = nc.dram_tensor("mask", [BPC, G, K], U8, kind="ExternalOutput").ap()

    NQ = BPC * Q          # 16000 query slots across the core's images
    QP = NQ // 128        # 125 queries per partition in phase-0 layout

    with tile.TileContext(nc) as tc, ExitStack() as ctx:
        dram = ctx.enter_context(tc.tile_pool(name="dram", bufs=1, space="DRAM"))
        const = ctx.enter_context(tc.tile_pool(name="const", bufs=1))
        prep = ctx.enter_context(tc.tile_pool(name="prep", bufs=1))
        rows = ctx.enter_context(tc.tile_pool(name="rows", bufs=2))
        work = ctx.enter_context(tc.tile_pool(name="work", bufs=2))
        tiny = ctx.enter_context(tc.tile_pool(name="tiny", bufs=2))
        outp = ctx.enter_context(tc.tile_pool(name="outp", bufs=2))

        # ---- phase 0: per-query area + sigmoid score for all 16 images ----
        pa_d = dram.tile([NQ], F32, tag="pa_d")
        sc_d = dram.tile([NQ], F32, tag="sc_d")

        # packed pred boxes: partition p holds queries [QP*p, QP*p+QP), 4 coords
        pbt = prep.tile([128, QP * 4], F32, tag="pbt")
        src = pb.rearrange("b q c -> (b q c)").rearrange("(p x) -> p x", p=128)
        nc.sync.dma_start(pbt[:], src)
        pv = pbt[:].rearrange("p (r c) -> p r c", c=4)
        dx = prep.tile([128, QP], F32, tag="dx")
        dy = prep.tile([128, QP], F32, tag="dy")
        pa = prep.tile([128, QP], F32, tag="pa")
        nc.vector.tensor_tensor(dx[:], pv[:, :, 2], pv[:, :, 0], Op.subtract)
        nc.vector.tensor_tensor(dy[:], pv[:, :, 3], pv[:, :, 1], Op.subtract)
        nc.vector.tensor_tensor(pa[:], dx[:], dy[:], Op.mult)
        nc.sync.dma_start(pa_d[:].rearrange("(p r) -> p r", p=128), pa[:])

        # target areas for all images: tgt boxes flat 16*300*4 = 19200 = 96*200
        ta_d = dram.tile([BPC * G], F32, tag="ta_d")
        tbt = prep.tile([96, 200], F32, tag="tbt")
        nc.sync.dma_start(
            tbt[:], tb.rearrange("b g c -> (b g c)").rearrange("(p x) -> p x", p=96)
        )
        tv = tbt[:].rearrange("p (r c) -> p r c", c=4)
        tdx = prep.tile([96, 50], F32, tag="tdx")
        tdy = prep.tile([96, 50], F32, tag="tdy")
        tar = prep.tile([96, 50], F32, tag="tar")
        nc.vector.tensor_tensor(tdx[:], tv[:, :, 2], tv[:, :, 0], Op.subtract)
        nc.vector.tensor_tensor(tdy[:], tv[:, :, 3], tv[:, :, 1], Op.subtract)
        nc.vector.tensor_tensor(tar[:], tdx[:], tdy[:], Op.mult)
        nc.sync.dma_start(ta_d[:].rearrange("(p r) -> p r", p=96), tar[:])

        # sigmoid(x) = 1 / (1 + exp(-x)); exp on ScalarE, exact-ish recip on DVE
        lg = prep.tile([128, QP], F32, tag="lg")
        nc.sync.dma_start(
            lg[:], pl.rearrange("b q c -> (b q c)").rearrange("(p x) -> p x", p=128)
        )
        ex = prep.tile([128, QP], F32, tag="ex")
        nc.scalar.activation(ex[:], lg[:], mybir.ActivationFunctionType.Exp,
                             scale=-1.0)
        w1 = prep.tile([128, QP], F32, tag="w1")
        nc.vector.tensor_scalar(w1[:], ex[:], 1.0, None, Op.add)
        sc = prep.tile([128, QP], F32, tag="sc")
        scr = prep.tile([128, QP], F32, tag="scr")
        nc.vector.reciprocal_approx_accurate(sc[:], w1[:], scr[:])
        nc.sync.dma_start(sc_d[:].rearrange("(p r) -> p r", p=128), sc[:])

        # ---- tie-break bias row: (Q - q) * 2^-40, identical on all partitions
        bias_i = const.tile([128, Q], I32, tag="bias_i")
        nc.gpsimd.iota(bias_i[:], pattern=[[-1, Q]], base=Q, channel_multiplier=0)
        bias_f = const.tile([128, Q], F32, tag="bias_f")
        nc.vector.tensor_scalar(bias_f[:], bias_i[:], BIAS_SCALE, None, Op.mult)

        # ---- main loop: per image, per g-chunk ----
        for b in [bb for _ in range(reps) for bb in range(BPC)]:
            r_px1 = rows.tile([128, Q], F32, tag="px1")
            r_py1 = rows.tile([128, Q], F32, tag="py1")
            r_px2 = rows.tile([128, Q], F32, tag="px2")
            r_py2 = rows.tile([128, Q], F32, tag="py2")
            r_pa = rows.tile([128, Q], F32, tag="pa")
            r_sc = rows.tile([128, Q], F32, tag="sc")
            nc.sync.dma_start(r_px1[:], pb[b, :, 0].partition_broadcast(128))
            nc.sync.dma_start(r_py1[:], pb[b, :, 1].partition_broadcast(128))
            nc.sync.dma_start(r_px2[:], pb[b, :, 2].partition_broadcast(128))
            nc.sync.dma_start(r_py2[:], pb[b, :, 3].partition_broadcast(128))
            nc.sync.dma_start(r_pa[:], pa_d[b * Q:(b + 1) * Q].partition_broadcast(128))
            nc.sync.dma_start(r_sc[:], sc_d[b * Q:(b + 1) * Q].partition_broadcast(128))

            for g0 in range(0, G, 128):
                P = min(128, G - g0)

                tsc = tiny.tile([P, 4], F32, tag="tsc")
                nc.sync.dma_start(tsc[:], tb[b, g0:g0 + P, :])
                tx1, ty1 = tsc[:, 0:1], tsc[:, 1:2]
                tx2, ty2 = tsc[:, 2:3], tsc[:, 3:4]
                ta = tiny.tile([P, 1], F32, tag="ta")
                nc.sync.dma_start(
                    ta[:],
                    ta_d[b * G + g0:b * G + g0 + P].rearrange("(p x) -> p x", x=1),
                )

                # w/h pre-relu overlap widths, one fused custom op each:
                # wxr = min(px2, tx2) - max(px1, tx1)
                wxr = work.tile([P, Q], F32, tag="wxr")
                nc.vector._custom_dve(wsub, out=wxr[:], in0=r_px2[:P],
                                      in1=r_px1[:P], s0=tx2, s1=tx1)
                wyr = work.tile([P, Q], F32, tag="wyr")
                nc.vector._custom_dve(wsub, out=wyr[:], in0=r_py2[:P],
                                      in1=r_py1[:P], s0=ty2, s1=ty1)
                # inter = relu(wxr) * wyr  (sign-exact; == ref where it matters)
                inter = work.tile([P, Q], F32, tag="inter")
                nc.vector.scalar_tensor_tensor(inter[:], wxr[:], 0.0, wyr[:],
                                               Op.max, Op.mult)
                # U = (pa + ta) - inter ; Up = U + 1e-7
                U = work.tile([P, Q], F32, tag="U")
                nc.vector.scalar_tensor_tensor(U[:], r_pa[:P], ta[:, 0:1], inter[:],
                                               Op.add, Op.subtract)
                Up = work.tile([P, Q], F32, tag="Up")
                nc.vector.tensor_scalar(Up[:], U[:], 1e-7, None, Op.add)
                # negd = 0.4*Up - inter  (valid <=> negd < 0)
                negd = work.tile([P, Q], F32, tag="negd")
                nc.vector.scalar_tensor_tensor(negd[:], Up[:], 0.4, inter[:],
                                               Op.mult, Op.subtract)
                # R ~= 1/Up to ~2 ULP
                R = work.tile([P, Q], F32, tag="R")
                rs = work.tile([P, Q], F32, tag="rs")
                nc.vector.reciprocal_approx_accurate(R[:], Up[:], rs[:])
                # m3 = (negd < 0) * ((inter * R) * score) + bias
                # plain tensor_tensor muls/adds run on gpsimd: DVE 1x ops
                # never contend with the shared port, so this is free overlap
                m1 = work.tile([P, Q], F32, tag="m1")
                nc.gpsimd.tensor_tensor(m1[:], inter[:], R[:], Op.mult)
                t1 = work.tile([P, Q], F32, tag="t1")
                nc.gpsimd.tensor_tensor(t1[:], m1[:], r_sc[:P], Op.mult)
                m2 = work.tile([P, Q], F32, tag="m2")
                nc.vector.scalar_tensor_tensor(m2[:], negd[:], 0.0, t1[:],
                                               Op.is_lt, Op.mult)
                m3 = work.tile([P, Q], F32, tag="m3")
                nc.gpsimd.tensor_tensor(m3[:], m2[:], bias_f[:P], Op.add)

                v8 = outp.tile([P, 8], F32, tag="v8")
                nc.vector.max(v8[:], m3[:])
                i8 = outp.tile([P, 8], U32, tag="i8")
                nc.vector.max_index(i8[:], v8[:], m3[:])

                # epilogue (gpsimd): exact zeros for padding slots + bool mask
                v4 = outp.tile([P, K], F32, tag="v4")
                nc.vector.scalar_tensor_tensor(v4[:], v8[:, 0:K], POS_THRESH,
                                               v8[:, 0:K], Op.is_gt, Op.mult)
                mk = outp.tile([P, K], U8, tag="mk")
                nc.vector.tensor_scalar(mk[:], v8[:, 0:K], POS_THRESH, None, Op.is_gt)

                nc.sync.dma_start(vals_o[b, g0:g0 + P, :], v4[:])
                nc.sync.dma_start(idxs_o[b, g0:g0 + P, :], i8[:, 0:K].bitcast(I32))
                nc.sync.dma_start(mask_o[b, g0:g0 + P, :], mk[:])

    nc.compile()
    return nc


_NC = None


def _get_nc():
    global _NC
    if _NC is None:
        _NC = _build_kernel()
    return _NC


def run(pred_logits, pred_boxes_xyxy, tgt_boxes_xyxy, **spmd_kwargs):
    nc = _get_nc()
    pred_logits = np.ascontiguousarray(np.asarray(pred_logits, dtype=np.float32))
    pred_boxes = np.ascontiguousarray(np.asarray(pred_boxes_xyxy, dtype=np.float32))
    tgt_boxes = np.ascontiguousarray(np.asarray(tgt_boxes_xyxy, dtype=np.float32))
    in_maps = [
        {
            "pred_logits": pred_logits[c * BPC:(c + 1) * BPC],
            "pred_boxes": pred_boxes[c * BPC:(c + 1) * BPC],
            "tgt_boxes": tgt_boxes[c * BPC:(c + 1) * BPC],
        }
        for c in range(NCORES)
    ]
    res = run_bass_kernel_spmd(nc, in_maps, list(range(NCORES)), **spmd_kwargs)
    vals = np.concatenate([res.results[c]["vals"] for c in range(NCORES)], axis=0)
    idxs = np.concatenate([res.results[c]["idxs"] for c in range(NCORES)], axis=0)
    mask = np.concatenate([res.results[c]["mask"] for c in range(NCORES)], axis=0)
    return (vals, idxs.astype(np.int32), mask.astype(bool)), res


def kernel(pred_logits, pred_boxes_xyxy, tgt_boxes_xyxy):
    (vals, idxs, mask), _ = run(pred_logits, pred_boxes_xyxy, tgt_boxes_xyxy)
    return vals, idxs, mask


# revision 18
# speedup vs baseline: 16.9088x; 16.9088x over previous
"""BinaryOneToManyMatcher (nms_detection) Trainium2 Bass kernel.

Computes, for B=128 images with Q=1000 predicted boxes and G=300 GT boxes:
  score = sigmoid(pred_logits)            [B,Q]
  iou   = pairwise IoU(pred, tgt)         [B,Q,G]
  gt    = score * iou * (iou > 0.4)       [B,Q,G]
  vals, idxs = top_k(gt over Q, k=4); mask = vals > 0

Sharding: pure data parallel, 16 images per NeuronCore across 8 cores.

Per-core layout: for each image, G on partitions (chunks of <=128) and Q on
the free dim (1000 wide).  Per-query rows (x1,y1,x2,y2,area,score) are
broadcast across partitions with stride-0 DMA; per-target values are [P,1]
per-partition scalars, so the whole IoU chain runs as fused
tensor_scalar / scalar_tensor_tensor ops on the Vector engine.

Top-4 uses the DVE Max8 instruction (top-8 per partition, descending) +
MaxIndex.  A strictly-decreasing per-q bias of scale 2^-40 is added to the
masked scores so every value in a row is distinct; this makes tie handling
exact: zero entries (invalid pairs) sort by ascending q, matching
jax.lax.top_k's lowest-index-first tie rule, and the bias is far below the
minimum positive score gap so positive ordering is unchanged.
"""

from contextlib import ExitStack

import numpy as np

import concourse.bass as bass
import concourse.tile as tile
from concourse import bacc, mybir
from concourse.bass_utils import run_bass_kernel_spmd

B, Q, G, K = 128, 1000, 300, 4
NCORES = 8
BPC = B // NCORES  # images per core

F32 = mybir.dt.float32
I32 = mybir.dt.int32
U32 = mybir.dt.uint32
U8 = mybir.dt.uint8
Op = mybir.AluOpType

BIAS_SCALE = float(2.0**-40)  # per-q tie-break bias scale
POS_THRESH = 1e-6  # separates real positives (>=3e-3) from bias values (<1e-9)


def _register_wsub():
    """Custom DVE op: out = min(in0, s0) - max(in1, s1) in one pass.

    Computes the overlap width rb-lt of the IoU kernel (normally a
    tensor_scalar max + a fused min/subtract = 2 DVE passes) in a single
    full-rate instruction.  Rounding matches the reference exactly: min/max
    are exact, one rounded subtract.
    """
    from concourse import dve_ops
    from concourse.dve_spec import Spec, Src0, Src1, C0, C1, minn, maxx, lower
    from concourse.dve_uop import DveOpSpec

    for op in dve_ops.OPS:
        if op.name == "WSUB_ANT":
            return op

    spec = Spec(
        body=minn(Src0, C0) - maxx(Src1, C1),
        reference=lambda in0, in1, s0, s1, imm2: (
            np.minimum(in0.astype(np.float32), s0) - np.maximum(in1, s1)
        ).astype(np.float32),
    )
    shas = {}
    for ver in ("v3", "v4"):
        try:
            uops = lower(spec, ver=ver)
            shas[ver] = DveOpSpec(
                name="WSUB_ANT", opcode=0, uops=uops, rd1_en=True
            ).sha(ver)
        except Exception:
            pass
    op = dve_ops.DveOp("WSUB_ANT", spec, subdim=False, uops_sha=shas)
    dve_ops.OPS.append(op)
    dve_ops.CUSTOM_DVE_SPECS[op.name] = spec
    dve_ops._SUB_OPCODE_FOR_NAME[op.name] = (
        max(dve_ops._SUB_OPCODE_FOR_NAME.values()) + 1
    )
    assert dve_ops._SUB_OPCODE_FOR_NAME[op.name] < 0x20
    return op


def _build_kernel(reps=1):
    wsub = _register_wsub()
    nc = bacc.Bacc("TRN2", target_bir_lowering=False, debug=False,
                   num_devices=NCORES)

    pl = nc.dram_tensor("pred_logits", [BPC, Q, 1], F32, kind="ExternalInput").ap()
    pb = nc.dram_tensor("pred_boxes", [BPC, Q, 4], F32, kind="ExternalInput").ap()
    tb = nc.dram_tensor("tgt_boxes", [BPC, G, 4], F32, kind="ExternalInput").ap()

    vals_o = nc.dram_tensor("vals", [BPC, G, K], F32, kind="ExternalOutput").ap()
    idxs_o = nc.dram_tensor("idxs", [BPC, G, K], I32, kind="ExternalOutput").ap()
    mask_o = nc.dram_tensor("mask", [BPC, G, K], U8, kind="ExternalOutput").ap()

    NQ = BPC * Q          # 16000 query slots across the core's images
    QP = NQ // 128        # 125 queries per partition in phase-0 layout

    with tile.TileContext(nc) as tc, ExitStack() as ctx:
        dram = ctx.enter_context(tc.tile_pool(name="dram", bufs=1, space="DRAM"))
        const = ctx.enter_context(tc.tile_pool(name="const", bufs=1))
        prep = ctx.enter_context(tc.tile_pool(name="prep", bufs=1))
        rows = ctx.enter_context(tc.tile_pool(name="rows", bufs=2))
        work = ctx.enter_context(tc.tile_pool(name="work", bufs=2))
        tiny = ctx.enter_context(tc.tile_pool(name="tiny", bufs=2))
        outp = ctx.enter_context(tc.tile_pool(name="outp", bufs=2))
        psum = ctx.enter_context(tc.tile_pool(name="psum", bufs=4, space="PSUM"))

        # ---- phase 0: build rowpack = [px1,py1,px2,py2,area,score] per image
        # (contiguous per-image 6*Q lines; broadcast later via PE ones-matmul,
        # NOT stride-0 DMA — partition-broadcast DMA is descriptor-bound at
        # ~96us per [128,1000] broadcast on HW)
        rowpack_d = dram.tile([BPC * 6 * Q], F32, tag="rowpack")
        rowview = rowpack_d[:].rearrange("(b c q) -> b c q", c=6, q=Q)
        PH = Q // QP  # partitions per image in phase-0 layout (8)

        def pack_row(j, tile_view):
            # tile_view: [128, QP] SBUF, partition 8b+ph = queries of image b
            for bb in range(BPC):
                nc.sync.dma_start(
                    rowview[bb, j, :].rearrange("(ph r) -> ph r", ph=PH),
                    tile_view[bb * PH:(bb + 1) * PH, :],
                )

        # packed pred boxes: partition p holds queries [QP*p, QP*p+QP), 4 coords
        pbt = prep.tile([128, QP * 4], F32, tag="pbt")
        src = pb.rearrange("b q c -> (b q c)").rearrange("(p x) -> p x", p=128)
        nc.sync.dma_start(pbt[:], src)
        pv = pbt[:].rearrange("p (r c) -> p r c", c=4)
        for c in range(4):
            pack_row(c, pv[:, :, c])
        dx = prep.tile([128, QP], F32, tag="dx")
        dy = prep.tile([128, QP], F32, tag="dy")
        pa = prep.tile([128, QP], F32, tag="pa")
        nc.vector.tensor_tensor(dx[:], pv[:, :, 2], pv[:, :, 0], Op.subtract)
        nc.vector.tensor_tensor(dy[:], pv[:, :, 3], pv[:, :, 1], Op.subtract)
        nc.vector.tensor_tensor(pa[:], dx[:], dy[:], Op.mult)
        pack_row(4, pa[:])

        # target areas for all images: tgt boxes flat 16*300*4 = 19200 = 96*200
        ta_d = dram.tile([BPC * G], F32, tag="ta_d")
        tbt = prep.tile([96, 200], F32, tag="tbt")
        nc.sync.dma_start(
            tbt[:], tb.rearrange("b g c -> (b g c)").rearrange("(p x) -> p x", p=96)
        )
        tv = tbt[:].rearrange("p (r c) -> p r c", c=4)
        tdx = prep.tile([96, 50], F32, tag="tdx")
        tdy = prep.tile([96, 50], F32, tag="tdy")
        tar = prep.tile([96, 50], F32, tag="tar")
        nc.vector.tensor_tensor(tdx[:], tv[:, :, 2], tv[:, :, 0], Op.subtract)
        nc.vector.tensor_tensor(tdy[:], tv[:, :, 3], tv[:, :, 1], Op.subtract)
        nc.vector.tensor_tensor(tar[:], tdx[:], tdy[:], Op.mult)
        nc.sync.dma_start(ta_d[:].rearrange("(p r) -> p r", p=96), tar[:])

        # sigmoid(x) = 1 / (1 + exp(-x)); exp on ScalarE, exact-ish recip on DVE
        lg = prep.tile([128, QP], F32, tag="lg")
        nc.sync.dma_start(
            lg[:], pl.rearrange("b q c -> (b q c)").rearrange("(p x) -> p x", p=128)
        )
        ex = prep.tile([128, QP], F32, tag="ex")
        nc.scalar.activation(ex[:], lg[:], mybir.ActivationFunctionType.Exp,
                             scale=-1.0)
        w1 = prep.tile([128, QP], F32, tag="w1")
        nc.vector.tensor_scalar(w1[:], ex[:], 1.0, None, Op.add)
        sc = prep.tile([128, QP], F32, tag="sc")
        scr = prep.tile([128, QP], F32, tag="scr")
        nc.vector.reciprocal_approx_accurate(sc[:], w1[:], scr[:])
        pack_row(5, sc[:])

        # ones row for PE-based partition broadcast
        ones = const.tile([1, 128], F32, tag="ones")
        nc.vector.memset(ones[:], 1.0)

        # ---- tie-break bias row: (Q - q) * 2^-40, identical on all partitions
        bias_i = const.tile([128, Q], I32, tag="bias_i")
        nc.gpsimd.iota(bias_i[:], pattern=[[-1, Q]], base=Q, channel_multiplier=0)
        bias_f = const.tile([128, Q], F32, tag="bias_f")
        nc.vector.tensor_scalar(bias_f[:], bias_i[:], BIAS_SCALE, None, Op.mult)

        # ---- main loop: per image, per g-chunk ----
        for b in [bb for _ in range(reps) for bb in range(BPC)]:
            r_px1 = rows.tile([128, Q], F32, tag="px1")
            r_py1 = rows.tile([128, Q], F32, tag="py1")
            r_px2 = rows.tile([128, Q], F32, tag="px2")
            r_py2 = rows.tile([128, Q], F32, tag="py2")
            r_pa = rows.tile([128, Q], F32, tag="pa")
            r_sc = rows.tile([128, Q], F32, tag="sc")
            # one contiguous 24KB line DMA, then PE ones-matmul broadcast
            # (bit-exact: 1.0*x single product) + ScalarE PSUM->SBUF copies
            line = rows.tile([1, 6 * Q], F32, tag="line")
            nc.sync.dma_start(
                line[:],
                rowpack_d[b * 6 * Q:(b + 1) * 6 * Q].rearrange("(a x) -> a x", a=1),
            )
            HB = 500  # psum bank-sized broadcast piece
            for j, rt in enumerate((r_px1, r_py1, r_px2, r_py2, r_pa, r_sc)):
                for h in range(Q // HB):
                    pt = psum.tile([128, HB], F32, tag="pt")
                    nc.tensor.matmul(pt[:], ones[:],
                                     line[0:1, j * Q + h * HB:j * Q + (h + 1) * HB],
                                     start=True, stop=True)
                    nc.scalar.copy(rt[:, h * HB:(h + 1) * HB], pt[:])

            for g0 in range(0, G, 128):
                P = min(128, G - g0)

                tsc = tiny.tile([P, 4], F32, tag="tsc")
                nc.sync.dma_start(tsc[:], tb[b, g0:g0 + P, :])
                tx1, ty1 = tsc[:, 0:1], tsc[:, 1:2]
                tx2, ty2 = tsc[:, 2:3], tsc[:, 3:4]
                ta = tiny.tile([P, 1], F32, tag="ta")
                nc.sync.dma_start(
                    ta[:],
                    ta_d[b * G + g0:b * G + g0 + P].rearrange("(p x) -> p x", x=1),
                )

                # w/h pre-relu overlap widths, one fused custom op each:
                # wxr = min(px2, tx2) - max(px1, tx1)
                wxr = work.tile([P, Q], F32, tag="wxr")
                nc.vector._custom_dve(wsub, out=wxr[:], in0=r_px2[:P],
                                      in1=r_px1[:P], s0=tx2, s1=tx1)
                wyr = work.tile([P, Q], F32, tag="wyr")
                nc.vector._custom_dve(wsub, out=wyr[:], in0=r_py2[:P],
                                      in1=r_py1[:P], s0=ty2, s1=ty1)
                # inter = relu(wxr) * wyr  (sign-exact; == ref where it matters)
                inter = work.tile([P, Q], F32, tag="inter")
                nc.vector.scalar_tensor_tensor(inter[:], wxr[:], 0.0, wyr[:],
                                               Op.max, Op.mult)
                # U = (pa + ta) - inter ; Up = U + 1e-7
                U = work.tile([P, Q], F32, tag="U")
                nc.vector.scalar_tensor_tensor(U[:], r_pa[:P], ta[:, 0:1], inter[:],
                                               Op.add, Op.subtract)
                Up = work.tile([P, Q], F32, tag="Up")
                nc.vector.tensor_scalar(Up[:], U[:], 1e-7, None, Op.add)
                # negd = 0.4*Up - inter  (valid <=> negd < 0)
                negd = work.tile([P, Q], F32, tag="negd")
                nc.vector.scalar_tensor_tensor(negd[:], Up[:], 0.4, inter[:],
                                               Op.mult, Op.subtract)
                # R ~= 1/Up to ~2 ULP
                R = work.tile([P, Q], F32, tag="R")
                rs = work.tile([P, Q], F32, tag="rs")
                nc.vector.reciprocal_approx_accurate(R[:], Up[:], rs[:])
                # m3 = (negd < 0) * ((inter * R) * score) + bias
                # plain tensor_tensor muls/adds run on gpsimd: DVE 1x ops
                # never contend with the shared port, so this is free overlap
                m1 = work.tile([P, Q], F32, tag="m1")
                nc.gpsimd.tensor_tensor(m1[:], inter[:], R[:], Op.mult)
                t1 = work.tile([P, Q], F32, tag="t1")
                nc.gpsimd.tensor_tensor(t1[:], m1[:], r_sc[:P], Op.mult)
                m2 = work.tile([P, Q], F32, tag="m2")
                nc.vector.scalar_tensor_tensor(m2[:], negd[:], 0.0, t1[:],
                                               Op.is_lt, Op.mult)
                m3 = work.tile([P, Q], F32, tag="m3")
                nc.gpsimd.tensor_tensor(m3[:], m2[:], bias_f[:P], Op.add)

                v8 = outp.tile([P, 8], F32, tag="v8")
                nc.vector.max(v8[:], m3[:])
                i8 = outp.tile([P, 8], U32, tag="i8")
                nc.vector.max_index(i8[:], v8[:], m3[:])

                # epilogue (gpsimd): exact zeros for padding slots + bool mask
                v4 = outp.tile([P, K], F32, tag="v4")
                nc.vector.scalar_tensor_tensor(v4[:], v8[:, 0:K], POS_THRESH,
                                               v8[:, 0:K], Op.is_gt, Op.mult)
                mk = outp.tile([P, K], U8, tag="mk")
                nc.vector.tensor_scalar(mk[:], v8[:, 0:K], POS_THRESH, None, Op.is_gt)

                nc.sync.dma_start(vals_o[b, g0:g0 + P, :], v4[:])
                nc.sync.dma_start(idxs_o[b, g0:g0 + P, :], i8[:, 0:K].bitcast(I32))
                nc.sync.dma_start(mask_o[b, g0:g0 + P, :], mk[:])

    nc.compile()
    return nc


_NC = None


def _get_nc():
    global _NC
    if _NC is None:
        _NC = _build_kernel()
    return _NC


def run(pred_logits, pred_boxes_xyxy, tgt_boxes_xyxy, **spmd_kwargs):
    nc = _get_nc()
    pred_logits = np.ascontiguousarray(np.asarray(pred_logits, dtype=np.float32))
    pred_boxes = np.ascontiguousarray(np.asarray(pred_boxes_xyxy, dtype=np.float32))
    tgt_boxes = np.ascontiguousarray(np.asarray(tgt_boxes_xyxy, dtype=np.float32))
    in_maps = [
        {
            "pred_logits": pred_logits[c * BPC:(c + 1) * BPC],
            "pred_boxes": pred_boxes[c * BPC:(c + 1) * BPC],
            "tgt_boxes": tgt_boxes[c * BPC:(c + 1) * BPC],
        }
        for c in range(NCORES)
    ]
    res = run_bass_kernel_spmd(nc, in_maps, list(range(NCORES)), **spmd_kwargs)
    vals = np.concatenate([res.results[c]["vals"] for c in range(NCORES)], axis=0)
    idxs = np.concatenate([res.results[c]["idxs"] for c in range(NCORES)], axis=0)
    mask = np.concatenate([res.results[c]["mask"] for c in range(NCORES)], axis=0)
    return (vals, idxs.astype(np.int32), mask.astype(bool)), res


def kernel(pred_logits, pred_boxes_xyxy, tgt_boxes_xyxy):
    (vals, idxs, mask), _ = run(pred_logits, pred_boxes_xyxy, tgt_boxes_xyxy)
    return vals, idxs, mask
